# revision 1
# baseline (speedup 1.0000x reference)
"""Trainium2 Bass kernel for nn_BaseLUTLayer (soft-LUT layer).

Math: out[b,o] = sum_k lut[o,k] * prod_j (bit_j(k) ? x[b,m(o,j)] : 1-x[b,m(o,j)])

Strategy (per core, batch-sharded 8 ways, 128 batch rows each):
  * odds transform: with w = 1-x, r = x/(1-x):
        out[b,o] = (prod_j w_j) * H,   H = successive halving of lut with
        T_new[k'] = T_lo[k'] + r_j * T_hi[k']   (6 levels, 2 DVE ops/elem)
  * layout: nodes on SBUF partitions (o_p = o % 128), free dims (k', b).
    lut tiles live per-partition (no replication); r/w values are gathered
    per (node, wire) with dma_gather using compile-time indices derived
    from `mapping` (host-known at trace time).
  * gather source: G[row(i)] = [w[:,i] (128 f32) | r[:,i] (128 f32)] built
    on-device (clamp, 1-x, reciprocal, PE transposes) and bounced via HBM.
"""

import numpy as np

import concourse.bass as bass
import concourse.mybir as mybir
from concourse import bacc
from concourse import tile
from concourse.masks import make_identity
from concourse.bass_utils import run_bass_kernel_spmd

P = 128
IN = 1024
OUT = 2048
NB = 6
B_FULL = 1024
N_CORES = 8
OHI = OUT // P  # 16
F32 = mybir.dt.float32
I16 = mybir.dt.int16
# clamp x <= 1 - 2^-18 so r = x/(1-x) <= 2^18 and r^6 stays well inside fp32
CLAMP = float(1.0 - 2.0**-18)

# chunks of the o_hi loop assigned to gpsimd instead of DVE (load balance:
# gpsimd 2-input elementwise is ~2x slower than DVE, so give it ~1/3)
GPSIMD_CHUNKS = ()
K_ACT = 28  # level-1 k'-slices on ScalarE; rest on DVE


def _mult():
    return mybir.AluOpType.mult


def _add():
    return mybir.AluOpType.add


def build_program():
    nc = bacc.Bacc("TRN2", target_bir_lowering=False, debug=False)

    xs = nc.dram_tensor("xs", [P, IN], F32, kind="ExternalInput").ap()
    gidx = nc.dram_tensor("gidx", [P, OUT * NB // 16], I16, kind="ExternalInput").ap()
    lutg = nc.dram_tensor("lutg", [P, OHI, 64], F32, kind="ExternalInput").ap()
    outs = nc.dram_tensor("outs", [P, OHI, P], F32, kind="ExternalOutput").ap()

    with tile.TileContext(nc) as tc:
        with (
            tc.tile_pool(name="consts", bufs=1) as consts,
            tc.tile_pool(name="main", bufs=1) as main,
            tc.tile_pool(name="zpool", bufs=5) as zpool,
            tc.tile_pool(name="tpool", bufs=3) as tpool,
            tc.tile_pool(name="spool", bufs=2) as spool,
            tc.tile_pool(name="dram", bufs=1, space="DRAM") as dpool,
        ):
            ident = consts.tile([P, P], F32)
            make_identity(nc, ident)

            gd = dpool.tile([P * (IN // P), 2 * P], F32)
            gd_warm = gd

            gidx_sb = consts.tile([P, OUT * NB // 16], I16)
            nc.sync.dma_start(gidx_sb, gidx)
            lutg_sb = consts.tile([P, OHI, 64], F32)
            nc.sync.dma_start(lutg_sb, lutg)

            # warm up the dma_gather ucode (IRAM load) before G is ready:
            # zero gd row 0, gather it 128 times into a scratch tile
            wzt = consts.tile([1, 2 * P], F32)
            nc.gpsimd.memset(wzt, 0.0)
            nc.sync.dma_start(gd_warm[0:1, :], wzt)
            widx = consts.tile([P, 8], I16)
            nc.gpsimd.memset(widx, 0)
            warm = consts.tile([P, 1, 2 * P], F32)
            nc.gpsimd.dma_gather(
                out_ap=warm,
                in_ap=gd_warm[0:1, :],
                idxs_ap=widx,
                num_idxs=P,
                num_idxs_reg=P,
                elem_size=2 * P,
            )

            # x shard, clamped; w = 1-x; r = x * (1/w)
            xt = main.tile([P, IN], F32)
            nc.sync.dma_start(xt, xs)
            nc.vector.tensor_scalar_min(xt, xt, CLAMP)
            wt = main.tile([P, IN], F32)
            nc.vector.tensor_scalar(
                out=wt, in0=xt, scalar1=-1.0, scalar2=1.0, op0=_mult(), op1=_add()
            )
            rw = main.tile([P, IN], F32)
            rt = main.tile([P, IN], F32)
            for q in range(4):
                qs = slice(q * (IN // 4), (q + 1) * (IN // 4))
                nc.vector.reciprocal(rw[:, qs], wt[:, qs])
                nc.vector.tensor_mul(rt[:, qs], xt[:, qs], rw[:, qs])

            # transpose w/r into G rows: G[(i%128)*8 + i//128] = [w[:,i] | r[:,i]]
            gsb = main.tile([P, IN // P, 2 * P], F32)
            with tc.tile_pool(name="psum_t", bufs=2, space="PSUM") as psum_t:
                for ih in range(IN // P):
                    pw = psum_t.tile([P, P], F32, tag="pt")
                    nc.tensor.transpose(pw, wt[:, ih * P : (ih + 1) * P], ident)
                    nc.scalar.copy(gsb[:, ih, 0:P], pw)
                    pr = psum_t.tile([P, P], F32, tag="pt")
                    nc.tensor.transpose(pr, rt[:, ih * P : (ih + 1) * P], ident)
                    nc.scalar.copy(gsb[:, ih, P : 2 * P], pr)

            gd_view = gd[:].rearrange("(p h) e -> p h e", h=IN // P)
            for ih in range(IN // P):
                nc.sync.dma_start(gd_view[:, ih, :], gsb[:, ih, :])

            # main loop over node chunks (128 nodes each)
            psum_cm = tc.tile_pool(name="psum", bufs=2, space="PSUM")
            psum = psum_cm.__enter__()
            idx_cols = NB * P // 16  # 48 idx columns per chunk

            # two-stage software pipeline: stage A (gather + monomial muls +
            # DMA pair-adds) for chunk c, then stage B (everything after the
            # DMA-adds) for chunk c-1 — keeps DVE's in-order queue from
            # stalling on the DMA-add completion.
            stash = {}

            def stage_a(c):
                z = zpool.tile([P, NB, 2 * P], F32, tag="z")
                if c == 0:
                    # split the first gather so L1/L2 (slots 0-2 = r5,r4,r3)
                    # can start before the whole chunk lands
                    half = idx_cols // 2
                    nc.gpsimd.dma_gather(
                        out_ap=z[:, 0 : NB // 2, :],
                        in_ap=gd[:],
                        idxs_ap=gidx_sb[:, 0:half],
                        num_idxs=NB * P // 2,
                        num_idxs_reg=NB * P // 2,
                        elem_size=2 * P,
                    )
                    nc.gpsimd.dma_gather(
                        out_ap=z[:, NB // 2 : NB, :],
                        in_ap=gd[:],
                        idxs_ap=gidx_sb[:, half:idx_cols],
                        num_idxs=NB * P // 2,
                        num_idxs_reg=NB * P // 2,
                        elem_size=2 * P,
                    )
                else:
                    nc.gpsimd.dma_gather(
                        out_ap=z,
                        in_ap=gd[:],
                        idxs_ap=gidx_sb[:, c * idx_cols : (c + 1) * idx_cols],
                        num_idxs=NB * P,
                        num_idxs_reg=NB * P,
                        elem_size=2 * P,
                    )
                # W = prod_j w_j (DVE)
                wp = spool.tile([P, 3, P], F32, tag="wp")
                nc.vector.tensor_mul(wp, z[:, 0:5:2, 0:P], z[:, 1:6:2, 0:P])
                wq = spool.tile([P, P], F32, tag="wq")
                nc.vector.tensor_mul(wq, wp[:, 0, :], wp[:, 1, :])
                nc.vector.tensor_mul(wq, wq, wp[:, 2, :])
                # level 1 on the (otherwise idle) Scalar engine:
                # t1[:, k', :] = r5 * lut1[k'] + lut0[k']  — lut entries are
                # per-partition scalars (scale/bias), r5 is the tensor input
                t1 = tpool.tile([P, 32, P], F32, tag="t1")
                r5t = z[:, 0, P : 2 * P]
                for kp in range(K_ACT):
                    nc.scalar.activation(
                        t1[:, kp, :],
                        r5t,
                        mybir.ActivationFunctionType.Identity,
                        bias=lutg_sb[:, c, kp : kp + 1],
                        scale=lutg_sb[:, c, 32 + kp : 33 + kp],
                    )
                kd = 32 - K_ACT
                if kd:
                    nc.vector.tensor_mul(
                        t1[:, K_ACT:32, :],
                        r5t[:, None, :].broadcast_to([P, kd, P]),
                        lutg_sb[:, c, 32 + K_ACT : 64][:, :, None].broadcast_to([P, kd, P]),
                    )
                    nc.vector.tensor_add(
                        t1[:, K_ACT:32, :],
                        t1[:, K_ACT:32, :],
                        lutg_sb[:, c, K_ACT:32][:, :, None].broadcast_to([P, kd, P]),
                    )
                # level 2 prod (DVE): prod2 = r4 * T1_hi ; t2 = T1_lo + prod2
                prod2 = tpool.tile([P, 16, P], F32, tag="pr16")
                nc.vector.tensor_mul(
                    prod2,
                    z[:, 1, P : 2 * P][:, None, :].broadcast_to([P, 16, P]),
                    t1[:, 16:32, :],
                )
                t2 = tpool.tile([P, 16, P], F32, tag="t2")
                nc.vector.tensor_add(t2, prod2, t1[:, 0:16, :])
                stash[c] = (z, t2, wq)

            def stage_b1(c):
                z, t2, wq = stash.pop(c)

                # level 3 (j=3, h=8): prod3 = r3*t2_hi (DVE);
                # acc[0:1024] = t2_lo + prod3 on TensorE; close R1 only
                pn8 = tpool.tile([P, 8, P], F32, tag="pr8")
                nc.vector.tensor_mul(
                    pn8,
                    z[:, 2, P : 2 * P][:, None, :].broadcast_to([P, 8, P]),
                    t2[:, 8:16, :],
                )
                t2f = t2[:].rearrange("p a b -> p (a b)")
                pn8f = pn8[:].rearrange("p a b -> p (a b)")
                acc = psum.tile([P, 8 * P], F32, tag="pacc")
                for s in range(2):
                    sl = slice(s * 512, (s + 1) * 512)
                    nc.tensor.matmul(
                        acc[:, sl], ident, t2f[:, sl], start=True, stop=False
                    )
                    nc.tensor.matmul(
                        acc[:, sl], ident, pn8f[:, sl], start=False, stop=(s == 1)
                    )
                stash[(c, "b2")] = (z, acc, wq)

            def stage_b2(c):
                z, acc, wq = stash.pop((c, "b2"))

                # level 4 (j=2, h=4): prod4 = r2*acc[4:8] (DVE);
                # acc[0:4] += prod4 (PE), closing R0
                pn4 = tpool.tile([P, 4, P], F32, tag="pr4")
                nc.vector.tensor_mul(
                    pn4,
                    z[:, 3, P : 2 * P][:, None, :].broadcast_to([P, 4, P]),
                    acc[:, 4 * P : 8 * P].rearrange("p (a b) -> p a b", b=P),
                )
                pn4f = pn4[:].rearrange("p a b -> p (a b)")
                nc.tensor.matmul(
                    acc[:, 0:512], ident, pn4f[:, 0:512], start=False, stop=True
                )

                # level 5 (j=1, h=2) from closed PSUM
                pn2 = tpool.tile([P, 2, P], F32, tag="pr2")
                nc.vector.tensor_mul(
                    pn2,
                    z[:, 4, P : 2 * P][:, None, :].broadcast_to([P, 2, P]),
                    acc[:, 2 * P : 4 * P].rearrange("p (a b) -> p a b", b=P),
                )
                t5 = tpool.tile([P, 2, P], F32, tag="t5")
                nc.vector.tensor_add(
                    t5, pn2, acc[:, 0 : 2 * P].rearrange("p (a b) -> p a b", b=P)
                )

                # level 6 (j=0, h=1)
                pn1 = tpool.tile([P, 1, P], F32, tag="pr1")
                nc.vector.tensor_mul(
                    pn1,
                    z[:, 5, P : 2 * P][:, None, :].broadcast_to([P, 1, P]),
                    t5[:, 1:2, :],
                )
                t6 = tpool.tile([P, 1, P], F32, tag="t6")
                nc.vector.tensor_add(t6, pn1, t5[:, 0:1, :])

                ot = spool.tile([P, P], F32, tag="ot")
                nc.vector.tensor_mul(ot, t6[:, 0, :], wq)
                nc.sync.dma_start(outs[:, c, :], ot)

            for c in range(OHI + 1):
                if c < OHI:
                    stage_a(c)
                if c >= 1:
                    stage_b1(c - 1)
                    stage_b2(c - 1)
            psum_cm.__exit__(None, None, None)

    # Bacc passes: event-sem generation (multi-wait lowering), auto library
    # loads for dma_gather, extended-InstISA byte packing, ...
    nc.compile()
    return nc


_CACHE: dict = {}


def _program():
    if "nc" not in _CACHE:
        _CACHE["nc"] = build_program()
    return _CACHE["nc"]


def make_inputs(x, lut_table, mapping):
    """Host-side input prep: shard x by batch, encode mapping as gather
    indices, split lut into node-on-partition lo/hi tiles."""
    x = np.ascontiguousarray(x, dtype=np.float32)
    lut_table = np.ascontiguousarray(lut_table, dtype=np.float32)
    mapping = np.asarray(mapping)

    # gather row of source column i: G row (i%128)*8 + i//128
    m3 = mapping.reshape(OHI, P, NB)  # [o_hi, o_p, j]
    rows = (m3 % P) * (IN // P) + (m3 // P)
    # t = (o_hi*NB + slot)*128 + o_p with slot = 5-j  ->  order (o_hi, 5-j, o_p)
    tvals = np.transpose(rows[:, :, ::-1], (0, 2, 1)).reshape(-1)
    gidx16 = tvals.reshape(-1, 16).T.astype(np.int16)  # [16, OUT*NB/16]
    gidx_arr = np.ascontiguousarray(np.tile(gidx16, (P // 16, 1)))

    lut3 = lut_table.reshape(OHI, P, 64).transpose(1, 0, 2)  # [o_p, o_hi, 64]
    lutg_arr = np.ascontiguousarray(lut3)

    in_maps = []
    for core in range(N_CORES):
        in_maps.append(
            {
                "xs": np.ascontiguousarray(x[core * P : (core + 1) * P]),
                "gidx": gidx_arr,
                "lutg": lutg_arr,
            }
        )
    return in_maps


def assemble_output(results):
    """results: list of 8 dicts with 'outs' [128, 16, 128] -> full [1024, 2048]."""
    parts = []
    for core in range(N_CORES):
        arr = results[core]["outs"]  # [o_p, o_hi, b]
        parts.append(np.ascontiguousarray(arr.transpose(2, 1, 0).reshape(P, OUT)))
    return np.concatenate(parts, axis=0)


def kernel_with_results(x, lut_table, mapping, **kwargs):
    nc = _program()
    in_maps = make_inputs(x, lut_table, mapping)
    res = run_bass_kernel_spmd(nc, in_maps, core_ids=list(range(N_CORES)), **kwargs)
    return assemble_output(res.results), res


def kernel(x, lut_table, mapping):
    out, _ = kernel_with_results(x, lut_table, mapping)
    return out


if __name__ == "__main__":
    rng = np.random.default_rng(0)
    x = rng.random((B_FULL, IN), dtype=np.float32)
    lut = rng.standard_normal((OUT, 64), dtype=np.float32)
    mp = rng.integers(0, IN, (OUT, NB), dtype=np.int32)
    out = kernel(x, lut, mp)
    print(out.shape, out.dtype)



# revision 2
# speedup vs baseline: 1.6367x; 1.6367x over previous
"""Trainium2 Bass kernel for nn_BaseLUTLayer (soft-LUT layer).

Math: out[b,o] = sum_k lut[o,k] * prod_j (bit_j(k) ? x[b,m(o,j)] : 1-x[b,m(o,j)])

v2 strategy (Mobius / multilinear-polynomial basis):
  * Host re-parameterizes the LUT into multilinear coefficients c[o, :]
    (per-bit transform (A,B) -> (A, B-A)), so that
        out[b,o] = sum_m c[o,m] * prod_{j: bit_j(m)} x[b, map(o,j)]
    evaluated by a 6-level halving tree with ADJACENT pairing:
        t_l[i] = t_{l-1}[2i] + t_{l-1}[2i+1] * x_{map(o, l-1)}
    No 1-x / reciprocals / w-product needed; all intermediates are
    bounded by sum|c| (~500), so the whole pipeline runs in fp16.
  * Sharding: 4-way over nodes x 2-way over batch -> per core B=512
    batch rows, 512 nodes = 4 chunks of 128 nodes (nodes on partitions).
  * Gather: host passes x^T as a [1024, 512] fp16 DRAM tensor per batch
    half; dma_gather pulls 6x128 rows (1KB each) per chunk. No on-device
    transposes at all.
  * Level 1 (32-wide, per-(o,kp) scalar FMA vs a shared tensor) runs
    mostly on the Scalar engine (activation: t1 = z0*c1[kp] + c0[kp]),
    remainder + levels 2-6 on DVE in fp16 2x mode. The c-coefficient
    broadcasts use duplicated-pair operands ([...,2] packed last dim) to
    keep every DVE operand unit-stride 2-byte => 2x throughput.
"""

import numpy as np

import concourse.bass as bass
import concourse.mybir as mybir
from concourse import bacc
from concourse import tile
from concourse.bass_utils import run_bass_kernel_spmd

P = 128
IN = 1024
OUT = 2048
NB = 6
B_FULL = 1024
N_CORES = 8

NODE_SHARDS = 4
BATCH_SHARDS = 2
B = B_FULL // BATCH_SHARDS          # 512 batch rows per core
NODES = OUT // NODE_SHARDS          # 512 nodes per core
OHI = NODES // P                    # 4 chunks of 128 nodes

F16 = mybir.dt.float16
F32 = mybir.dt.float32
I16 = mybir.dt.int16

# level-1 kp slices per 16-wide half handled by DVE (rest on ScalarE)
KDVE = 2
IDXC = NB * P // 16                 # gidx columns per chunk (48)


def build_program():
    nc = bacc.Bacc("TRN2", target_bir_lowering=False, debug=False)

    gsrc = nc.dram_tensor("gsrc", [IN, B], F16, kind="ExternalInput").ap()
    gidx = nc.dram_tensor("gidx", [P, OHI * IDXC], I16, kind="ExternalInput").ap()
    c0f = nc.dram_tensor("c0f", [P, OHI, 32], F32, kind="ExternalInput").ap()
    c1f = nc.dram_tensor("c1f", [P, OHI, 32], F32, kind="ExternalInput").ap()
    c0d = nc.dram_tensor("c0d", [P, OHI, 32, 2], F16, kind="ExternalInput").ap()
    c1d = nc.dram_tensor("c1d", [P, OHI, 32, 2], F16, kind="ExternalInput").ap()
    outs = nc.dram_tensor("outs", [P, OHI, B], F16, kind="ExternalOutput").ap()

    mul = mybir.AluOpType.mult
    add = mybir.AluOpType.add
    ident_fn = mybir.ActivationFunctionType.Identity

    with tile.TileContext(nc) as tc:
        with (
            tc.tile_pool(name="consts", bufs=1) as consts,
            tc.tile_pool(name="zpool", bufs=3) as zpool,
            tc.tile_pool(name="t1pool", bufs=2) as t1pool,
            tc.tile_pool(name="tpool", bufs=2) as tpool,
        ):
            gidx_sb = consts.tile([P, OHI * IDXC], I16)
            nc.sync.dma_start(gidx_sb, gidx)
            c0f_sb = consts.tile([P, OHI, 32], F32)
            nc.sync.dma_start(c0f_sb, c0f)
            c1f_sb = consts.tile([P, OHI, 32], F32)
            nc.sync.dma_start(c1f_sb, c1f)
            c0d_sb = consts.tile([P, OHI, 32, 2], F16)
            nc.sync.dma_start(c0d_sb, c0d)
            c1d_sb = consts.tile([P, OHI, 32, 2], F16)
            nc.sync.dma_start(c1d_sb, c1d)

            stash = {}

            def stage_a(c):
                z = zpool.tile([P, NB, B], F16, tag="z")
                nc.gpsimd.dma_gather(
                    out_ap=z,
                    in_ap=gsrc,
                    idxs_ap=gidx_sb[:, c * IDXC : (c + 1) * IDXC],
                    num_idxs=NB * P,
                    num_idxs_reg=NB * P,
                    elem_size=B,
                )
                z0 = z[:, 0, :]
                # level 1: t1[o, kp, b] = c0[o,kp] + c1[o,kp] * z0[o,b]
                t1 = t1pool.tile([P, 32, B], F16, tag="t1")
                z0d = z0.rearrange("p (b2 two) -> p b2 two", two=2)
                for h in range(2):
                    k0 = h * 16
                    # DVE share: last KDVE kp of the half (dup-pair trick, 2x)
                    kd0 = k0 + 16 - KDVE
                    t1v = t1[:, kd0 : k0 + 16, :].rearrange(
                        "p k (b2 two) -> p k b2 two", two=2
                    )
                    in_z = z0d[:, None, :, :].broadcast_to([P, KDVE, B // 2, 2])
                    nc.vector.tensor_mul(
                        t1v,
                        in_z,
                        c1d_sb[:, c, kd0 : k0 + 16, None, :].broadcast_to(
                            [P, KDVE, B // 2, 2]
                        ),
                    )
                    nc.vector.tensor_add(
                        t1v,
                        t1v,
                        c0d_sb[:, c, kd0 : k0 + 16, None, :].broadcast_to(
                            [P, KDVE, B // 2, 2]
                        ),
                    )
                    # ScalarE share: fused scale/bias FMA per kp slice
                    for kp in range(k0, kd0):
                        nc.scalar.activation(
                            t1[:, kp, :],
                            z0,
                            ident_fn,
                            bias=c0f_sb[:, c, kp : kp + 1],
                            scale=c1f_sb[:, c, kp : kp + 1],
                        )
                stash[c] = (z, t1)

            def stage_b(c):
                z, t1 = stash.pop(c)
                t5 = tpool.tile([P, 2, B], F16, tag="t5")
                for h in range(2):
                    k0 = h * 16
                    # level 2: 16 -> 8
                    p2 = tpool.tile([P, 8, B], F16, tag="p2")
                    nc.vector.tensor_mul(
                        p2,
                        z[:, 1, None, :].broadcast_to([P, 8, B]),
                        t1[:, k0 + 1 : k0 + 16 : 2, :],
                    )
                    t2 = tpool.tile([P, 8, B], F16, tag="t2")
                    nc.vector.tensor_add(t2, t1[:, k0 : k0 + 16 : 2, :], p2)
                    # level 3: 8 -> 4
                    p3 = tpool.tile([P, 4, B], F16, tag="p3")
                    nc.vector.tensor_mul(
                        p3, z[:, 2, None, :].broadcast_to([P, 4, B]), t2[:, 1::2, :]
                    )
                    t3 = tpool.tile([P, 4, B], F16, tag="t3")
                    nc.vector.tensor_add(t3, t2[:, 0::2, :], p3)
                    # level 4: 4 -> 2
                    p4 = tpool.tile([P, 2, B], F16, tag="p4")
                    nc.vector.tensor_mul(
                        p4, z[:, 3, None, :].broadcast_to([P, 2, B]), t3[:, 1::2, :]
                    )
                    t4 = tpool.tile([P, 2, B], F16, tag="t4")
                    nc.vector.tensor_add(t4, t3[:, 0::2, :], p4)
                    # level 5: 2 -> 1
                    p5 = tpool.tile([P, B], F16, tag="p5")
                    nc.vector.tensor_mul(p5, z[:, 4, :], t4[:, 1, :])
                    nc.vector.tensor_add(t5[:, h, :], t4[:, 0, :], p5)
                # level 6: combine halves
                p6 = tpool.tile([P, B], F16, tag="p6")
                nc.vector.tensor_mul(p6, z[:, 5, :], t5[:, 1, :])
                ot = tpool.tile([P, B], F16, tag="ot")
                nc.vector.tensor_add(ot, t5[:, 0, :], p6)
                nc.sync.dma_start(outs[:, c, :], ot)

            for c in range(OHI + 1):
                if c < OHI:
                    stage_a(c)
                if c >= 1:
                    stage_b(c - 1)

    nc.compile()
    return nc


_CACHE: dict = {}


def _program():
    if "nc" not in _CACHE:
        _CACHE["nc"] = build_program()
    return _CACHE["nc"]


def _mobius(lut_table):
    """Per-bit (A,B) -> (A, B-A): c[o,m] = coefficient of
    prod_{j: bit_j(m)=1} x_{map(o,j)} in the multilinear expansion."""
    c = lut_table.astype(np.float64).reshape(OUT, *(2,) * NB)
    for ax in range(1, NB + 1):
        a = np.take(c, 0, axis=ax)
        b = np.take(c, 1, axis=ax)
        c = np.stack([a, b - a], axis=ax)
    return c.reshape(OUT, 1 << NB)


def make_inputs(x, lut_table, mapping):
    x = np.asarray(x, dtype=np.float32)
    lut_table = np.asarray(lut_table, dtype=np.float32)
    mapping = np.asarray(mapping)

    c = _mobius(lut_table)  # [OUT, 64], float64
    c0 = c[:, 0::2]  # even entries -> bias   [OUT, 32]
    c1 = c[:, 1::2]  # odd entries  -> scale  [OUT, 32]

    # per node-shard tiles: node o = ns*NODES + chunk*P + o_p
    c0r = c0.reshape(NODE_SHARDS, OHI, P, 32)
    c1r = c1.reshape(NODE_SHARDS, OHI, P, 32)

    # gather indices: chunk-local position t = slot*128 + o_p, value =
    # mapping[o, slot]; wrapped into 16 partitions, tiled to 128
    m3 = mapping.reshape(NODE_SHARDS, OHI, P, NB)  # [ns, chunk, o_p, slot]
    tvals = np.transpose(m3, (0, 1, 3, 2)).reshape(NODE_SHARDS, -1)  # (chunk,slot,o_p)
    gidx_arrs = []
    for ns in range(NODE_SHARDS):
        g16 = tvals[ns].reshape(-1, 16).T.astype(np.int16)  # [16, OHI*IDXC]
        gidx_arrs.append(np.ascontiguousarray(np.tile(g16, (P // 16, 1))))

    gsrc_arrs = []
    for hb in range(BATCH_SHARDS):
        xh = x[hb * B : (hb + 1) * B]  # [B, IN]
        gsrc_arrs.append(np.ascontiguousarray(xh.T.astype(np.float16)))

    in_maps = []
    for core in range(N_CORES):
        ns, hb = core // BATCH_SHARDS, core % BATCH_SHARDS
        c0t = np.ascontiguousarray(np.transpose(c0r[ns], (1, 0, 2)))  # [P, OHI, 32]
        c1t = np.ascontiguousarray(np.transpose(c1r[ns], (1, 0, 2)))
        in_maps.append(
            {
                "gsrc": gsrc_arrs[hb],
                "gidx": gidx_arrs[ns],
                "c0f": c0t.astype(np.float32),
                "c1f": c1t.astype(np.float32),
                "c0d": np.ascontiguousarray(
                    np.repeat(c0t.astype(np.float16)[..., None], 2, axis=-1)
                ),
                "c1d": np.ascontiguousarray(
                    np.repeat(c1t.astype(np.float16)[..., None], 2, axis=-1)
                ),
            }
        )
    return in_maps


def assemble_output(results):
    """results: 8 dicts with 'outs' [P, OHI, B] fp16 -> full [B_FULL, OUT] f32."""
    full = np.empty((B_FULL, OUT), dtype=np.float32)
    for core in range(N_CORES):
        ns, hb = core // BATCH_SHARDS, core % BATCH_SHARDS
        arr = np.asarray(results[core]["outs"])  # [o_p, chunk, b]
        blk = arr.astype(np.float32).transpose(2, 1, 0).reshape(B, NODES)
        full[hb * B : (hb + 1) * B, ns * NODES : (ns + 1) * NODES] = blk
    return full


def kernel_with_results(x, lut_table, mapping, **kwargs):
    nc = _program()
    in_maps = make_inputs(x, lut_table, mapping)
    res = run_bass_kernel_spmd(nc, in_maps, core_ids=list(range(N_CORES)), **kwargs)
    return assemble_output(res.results), res


def kernel(x, lut_table, mapping):
    out, _ = kernel_with_results(x, lut_table, mapping)
    return out


if __name__ == "__main__":
    rng = np.random.default_rng(0)
    x = rng.random((B_FULL, IN), dtype=np.float32)
    lut = rng.standard_normal((OUT, 64), dtype=np.float32)
    mp = rng.integers(0, IN, (OUT, NB), dtype=np.int32)
    out = kernel(x, lut, mp)
    print(out.shape, out.dtype)
